# revision 10
# baseline (speedup 1.0000x reference)
"""Distributed causal multi-head attention block (GPT-2 style) for 8 TRN2 NeuronCores.

Sharding: data-parallel over batch (4 pairs of cores) x tensor-parallel over
heads (2 groups of 8 heads). Core c handles batch c//2, head-group c%2.

v4 reorganization vs v3:
  - ~33 junk matmuls on memset SBUF data at t=0 warm the PE/HAM clock gate
    through the ~9us DMA-bootstrap dead time, so real matmuls run at 2.4 GHz.
  - A tiny warmup AllReduce flushes the collective-stream barrier early.
  - Attention runs per head-PAIR: the two heads of a feature tile (partition
    rows 0:64 / 64:128) issue score matmuls back-to-back at row groups (0,0)
    and (64,0), so both K=64 matmuls stream concurrently through the PE.
  - Causal masking is folded into the scores PSUM via an identity-matmul that
    accumulates -240 above the diagonal BEFORE exp (exp then yields ~0), so
    the DVE is out of the scores->exp->PV critical chain.
  - One ACT exp per (pair, k-block): strided [128, 2, N] over both heads.
  - Head-major schedule: each (pair, half) finishes and immediately stages its
    aT rows for a pair-AllReduce(add); the partner's rows are recovered
    rank-agnostically as (sum - mine) on GpSimd. c_proj contracts own rows
    straight from SBUF and partner rows from the recovered copies, with
    host-reordered Wproj rows [own 512 | partner 512]. Only the last 128KB
    exchange sits near the tail, and its chunk-3 contribution uses the
    (sum*W + mine*(-W)) trick to skip the subtract latency.
  - c_proj per out-tile runs in two PSUM sessions (bias+chunks 0,1 early,
    chunks 2,3 late) joined by an SBUF f32 partial, so early proj matmuls
    interleave with late attention pairs on a 2-bank rotation.
"""

import numpy as np
import ml_dtypes

import concourse.bass as bass
import concourse.mybir as mybir
import concourse.tile as tile
from concourse import bacc
from concourse.bass_utils import run_bass_kernel_spmd
from concourse.masks import make_identity, make_lower_triangular

F32 = mybir.dt.float32
BF16 = mybir.dt.bfloat16
AF = mybir.ActivationFunctionType
ALU = mybir.AluOpType

P = 128
S = 1024          # sequence length
NX = 1024         # model width
D = 64            # head dim
H_LOC = 8         # heads per core
FEAT = H_LOC * D  # 512 local attention features
NKC = NX // P     # 8 contraction chunks
NST = S // P      # 8 sequence tiles
VW = D + 1        # v block width incl. ones column (65)
N_JUNK = 8        # warmup matmuls to open the HAM clock gate early

PAIRS = [[0, 1], [2, 3], [4, 5], [6, 7]]


def build():
    nc = bacc.Bacc(num_devices=8)
    xT = nc.dram_tensor("xT", [NX, S], BF16, kind="ExternalInput")
    wqkv = nc.dram_tensor("wqkv", [NX, 3 * FEAT], BF16, kind="ExternalInput")
    bqk_t = nc.dram_tensor("bqk_t", [P, 8], F32, kind="ExternalInput")
    bv_r = nc.dram_tensor("bv_r", [1, FEAT], BF16, kind="ExternalInput")
    wproj = nc.dram_tensor("wproj", [NX, FEAT], BF16, kind="ExternalInput")
    wpn3 = nc.dram_tensor("wpn3", [P, FEAT], BF16, kind="ExternalInput")
    bp_r = nc.dram_tensor("bp_r", [1, FEAT], BF16, kind="ExternalInput")
    out = nc.dram_tensor("out", [S, FEAT], F32, kind="ExternalOutput")

    with tile.TileContext(nc) as tc:
        with (
            tc.tile_pool(name="pt", bufs=8) as ptp,            # P^T pair blocks
            tc.tile_pool(name="small", bufs=2) as small,       # recip vectors
            tc.tile_pool(name="outp", bufs=3) as outp,         # out f32 tiles
            tc.tile_pool(name="dram", bufs=1, space="DRAM") as dram,
            tc.tile_pool(name="resident", bufs=1) as res,
        ):
            # ---- resident SBUF tensors ----
            xT_all = res.tile([P, NKC * S], BF16, tag="xT_all")
            wqkv_sb = res.tile([P, NKC * 3 * FEAT], BF16, tag="wqkv_sb")
            qkT_all = res.tile([P, 8 * S], BF16, tag="qkT_all")   # qT(0..3)|kT(4..7)
            v_sb = res.tile([P, NST * H_LOC * VW], BF16, tag="v_sb")
            aT_loc = res.tile([P, 4 * S], BF16, tag="aT_loc")     # my 512 feats
            blk0_sb = res.tile([P, 4 * S], BF16, tag="blk0_sb")   # gathered rank0
            blk1_sb = res.tile([P, 4 * S], BF16, tag="blk1_sb")   # gathered rank1
            part_sb = res.tile([P, 3 * S], BF16, tag="part_sb")   # partner (p<3)
            wp_sb = res.tile([P, NKC * FEAT], BF16, tag="wp_sb")  # [own|partner]
            wpn3_sb = res.tile([P, FEAT], BF16, tag="wpn3_sb")    # -W partner ch3
            partial_sb = res.tile([P, 8 * FEAT], F32, tag="partial_sb")
            bias_sb = res.tile([P, 8], F32, tag="bias_sb")
            bv_row = res.tile([1, FEAT], BF16, tag="bv_row")
            bp_row = res.tile([1, FEAT], BF16, tag="bp_row")
            ones_row = res.tile([1, P], BF16, tag="ones_row")
            junk_sb = res.tile([P, 512], BF16, tag="junk_sb")
            iden = res.tile([P, P], BF16, tag="iden")
            mask_add = res.tile([P, P], BF16, tag="mask_add")

            nc.vector.memset(ones_row[:], 1.0)
            nc.vector.memset(junk_sb[:], 0.001)
            nc.vector.memset(v_sb[:], 1.0)
            make_identity(nc, iden[:])
            make_lower_triangular(nc, mask_add[:], val=-240.0, diag=False)

            # ---- input stream. sync queue: x/w chunks (critical path) first,
            # then wproj + wpn3, then collective-sum reloads, then late out
            # tiles. gpsimd queue: biases, warmup-cc staging, per-half
            # broadcasts / staging / subs, early out tiles. ----
            for kc in range(NKC):
                xs = slice(kc * P, (kc + 1) * P)
                if kc < 2:
                    nc.sync.dma_start(
                        wqkv_sb[:, kc * 3 * FEAT : kc * 3 * FEAT + 640],
                        wqkv[xs, 0:640],
                    )
                    nc.sync.dma_start(
                        xT_all[:, kc * S : kc * S + 512], xT[xs, 0:512]
                    )
                    nc.sync.dma_start(
                        xT_all[:, kc * S + 512 : (kc + 1) * S], xT[xs, 512:1024]
                    )
                    nc.sync.dma_start(
                        wqkv_sb[:, kc * 3 * FEAT + 640 : (kc + 1) * 3 * FEAT],
                        wqkv[xs, 640:1536],
                    )
                else:
                    nc.sync.dma_start(xT_all[:, kc * S : (kc + 1) * S], xT[xs, :])
                    nc.sync.dma_start(
                        wqkv_sb[:, kc * 3 * FEAT : (kc + 1) * 3 * FEAT],
                        wqkv[xs, :],
                    )
            # warmup collective first on the gpsimd queue: DRAM->DRAM staging
            # with no compute deps, so the cc-stream barrier runs during boot
            cc_w_in = dram.tile([1, 8], F32, name="cc_w_in")
            cc_w_out = dram.tile([2, 8], F32, name="cc_w_out")
            nc.gpsimd.dma_start(cc_w_in[:, :], bqk_t[0:1, 0:8])
            nc.gpsimd.collective_compute(
                "AllGather", ALU.bypass, replica_groups=PAIRS,
                ins=[cc_w_in[:].opt()], outs=[cc_w_out[:].opt()],
            )
            nc.gpsimd.dma_start(bias_sb[:], bqk_t[:, :])
            nc.gpsimd.dma_start(bv_row[:], bv_r[:, :])
            nc.gpsimd.dma_start(bp_row[:], bp_r[:, :])
            for fc in range(NKC):
                nc.sync.dma_start(
                    wp_sb[:, fc * FEAT : (fc + 1) * FEAT],
                    wproj[fc * P : (fc + 1) * P, :],
                )
            nc.sync.dma_start(wpn3_sb[:], wpn3[:, :])

            # ---- per-(pair,half) collective staging ----
            cc_in = [dram.tile([P, 512], BF16, name=f"cc_in{u}") for u in range(8)]
            cc_out = [dram.tile([2 * P, 512], BF16, name=f"cc_out{u}") for u in range(8)]

            # ---- qkv group helpers ----
            def qk_mm(ps, ft, half, kc):
                nc.tensor.matmul(
                    ps[:],
                    wqkv_sb[:, kc * 3 * FEAT + ft * P : kc * 3 * FEAT + (ft + 1) * P],
                    xT_all[:, kc * S + half * 512 : kc * S + (half + 1) * 512],
                    start=(kc == 0),
                    stop=(kc == NKC - 1),
                )

            def qk_consume(ps, ft, half):
                nc.vector.tensor_scalar_add(
                    out=qkT_all[:, ft * S + half * 512 : ft * S + (half + 1) * 512],
                    in0=ps[:],
                    scalar1=bias_sb[:, ft : ft + 1],
                )

            def v_bias(ps):
                nc.tensor.matmul(ps[:], ones_row[:, 0:P], bv_row[:], start=True, stop=False)

            def v_mm(ps, st, kc):
                nc.tensor.matmul(
                    ps[:],
                    xT_all[:, kc * S + st * P : kc * S + (st + 1) * P],
                    wqkv_sb[:, kc * 3 * FEAT + 1024 : kc * 3 * FEAT + 1536],
                    start=False,
                    stop=(kc == NKC - 1),
                )

            def v_consume(ps, st):
                base = st * H_LOC * VW
                dst = v_sb[:, base : base + H_LOC * VW].rearrange(
                    "p (h w) -> p h w", h=H_LOC
                )[:, :, 0:D]
                src = ps[:].rearrange("p (h d) -> p h d", h=H_LOC)
                nc.vector.tensor_copy(out=dst, in_=src)

            # ---- wave 1: junk warmup + 8 groups fed in DMA-arrival order ----
            W1_QK = [(0, 0), (0, 1), (4, 0), (4, 1)]
            W1_V = [0, 1, 2, 3]
            with tc.tile_pool(name="ps_w1", bufs=8, space="PSUM") as psw:
                junk_ps = psw.tile([P, 512], F32, name="junk_ps", tag="w1")
                for _ in range(N_JUNK):
                    nc.tensor.matmul(
                        junk_ps[:], junk_sb[:, 0:P], junk_sb[:, 0:512],
                        start=True, stop=True,
                    )
                w1ps = {}
                for ft, half in W1_QK:
                    w1ps[("qk", ft, half)] = psw.tile(
                        [P, 512], F32, name=f"w1qk{ft}{half}", tag="w1"
                    )
                for st in W1_V:
                    ps = psw.tile([P, 512], F32, name=f"w1v{st}", tag="w1")
                    w1ps[("v", st)] = ps
                    v_bias(ps)
                for kc in range(NKC):
                    for ft, half in [(0, 0), (4, 0), (0, 1), (4, 1)]:
                        qk_mm(w1ps[("qk", ft, half)], ft, half, kc)
                    for st in W1_V:
                        v_mm(w1ps[("v", st)], st, kc)
                for ft, half in W1_QK:
                    qk_consume(w1ps[("qk", ft, half)], ft, half)
                for st in W1_V:
                    v_consume(w1ps[("v", st)], st)

            # ---- attention + remaining qkv + proj, interleaved ----
            with (
                tc.tile_pool(name="ps_sc", bufs=2, space="PSUM") as ps_sc,
                tc.tile_pool(name="ps_pa", bufs=1, space="PSUM") as ps_pa,
                tc.tile_pool(name="ps_sm", bufs=2, space="PSUM") as ps_sm,
            ):
                def qkT_tile(ft):
                    for half in range(2):
                        ps = ps_sm.tile([P, 512], F32, name="ps_qk", tag="sm")
                        for kc in range(NKC):
                            qk_mm(ps, ft, half, kc)
                        qk_consume(ps, ft, half)

                def v_tile(st):
                    ps = ps_sm.tile([P, 512], F32, name="ps_v", tag="sm")
                    v_bias(ps)
                    for kc in range(NKC):
                        v_mm(ps, st, kc)
                    v_consume(ps, st)

                def sub_unit(u):
                    # partner chunk p half qh = (block0 + block1) - my rows
                    pu, qu = divmod(u, 2)
                    col = pu * S + qu * 512
                    nc.gpsimd.tensor_tensor(
                        out=part_sb[:, col : col + 512],
                        in0=blk0_sb[:, col : col + 512],
                        in1=blk1_sb[:, col : col + 512],
                        op=ALU.add,
                    )
                    nc.gpsimd.tensor_tensor(
                        out=part_sb[:, col : col + 512],
                        in0=part_sb[:, col : col + 512],
                        in1=aT_loc[:, col : col + 512],
                        op=ALU.subtract,
                    )

                def attn_half(pair, qh):
                    nj = 4 * qh + 4
                    kcol = (4 + pair) * S
                    qbase = pair * S + qh * 512
                    u = pair * 2 + qh
                    pt_blocks = []
                    for j in range(nj):
                        dloc = j - 4 * qh
                        coff = max(dloc, 0) * P
                        diag = dloc >= 0
                        ps = ps_sc.tile([P, 1024], F32, name="ps_s", tag="sc")
                        ptb = ptp.tile([P, 1024], BF16, name="ptb", tag="pt")
                        for hh in range(2):
                            nc.tensor.matmul(
                                ps[:, hh * 512 + coff : hh * 512 + 512],
                                qkT_all[hh * D : (hh + 1) * D,
                                        kcol + j * P : kcol + (j + 1) * P],
                                qkT_all[hh * D : (hh + 1) * D,
                                        qbase + coff : qbase + 512],
                                start=True,
                                stop=not diag,
                            )
                        if diag:
                            for hh in range(2):
                                nc.tensor.matmul(
                                    ps[:, hh * 512 + coff : hh * 512 + coff + P],
                                    iden[:, 0:P],
                                    mask_add[:, 0:P],
                                    start=False,
                                    stop=True,
                                    skip_group_check=True,
                                )
                        pr = ps[:].rearrange("p (b n) -> p b n", b=2)[:, :, coff:512]
                        tr = ptb[:].rearrange("p (b n) -> p b n", b=2)[:, :, coff:512]
                        nc.scalar.activation(out=tr, in_=pr, func=AF.Exp, scale=0.125)
                        pt_blocks.append((ptb, coff))
                    psa = ps_pa.tile([P, 1024], F32, name="psa", tag="pa")
                    for j, (ptb, coff) in enumerate(pt_blocks):
                        for hh in range(2):
                            h = 2 * pair + hh
                            nc.tensor.matmul(
                                psa[:VW, hh * 512 + coff : hh * 512 + 512],
                                v_sb[:, j * H_LOC * VW + h * VW
                                     : j * H_LOC * VW + (h + 1) * VW],
                                ptb[:, hh * 512 + coff : hh * 512 + 512],
                                start=(j == 0),
                                stop=(j == nj - 1),
                            )
                    # normalize: recip of denominator row, broadcast, scale
                    db = small.tile([1, 1024], F32, tag="db")
                    nc.vector.tensor_copy(out=db[:], in_=psa[D : D + 1, 0:1024])
                    rc = small.tile([1, 1024], F32, tag="rc")
                    nc.vector.reciprocal_approx_fast(rc[:], db[:])
                    bcs = small.tile([D, 1024], F32, tag="bcs")
                    nc.gpsimd.partition_broadcast(bcs[:], rc[:])
                    acol = pair * S + qh * 512
                    for hh in range(2):
                        nc.vector.tensor_tensor(
                            out=aT_loc[hh * D : (hh + 1) * D, acol : acol + 512],
                            in0=bcs[:, hh * 512 : (hh + 1) * 512],
                            in1=psa[0:D, hh * 512 : (hh + 1) * 512],
                            op=ALU.mult,
                        )
                    # stage + pair AllGather of this half-chunk
                    nc.gpsimd.dma_start(cc_in[u][:, :], aT_loc[:, acol : acol + 512])
                    nc.gpsimd.collective_compute(
                        "AllGather", ALU.bypass, replica_groups=PAIRS,
                        ins=[cc_in[u][:].opt()], outs=[cc_out[u][:].opt()],
                    )
                    nc.sync.dma_start(blk0_sb[:, acol : acol + 512], cc_out[u][0:P, :])
                    nc.sync.dma_start(blk1_sb[:, acol : acol + 512], cc_out[u][P : 2 * P, :])
                    # partner recovery for unit u-2 (chunks 0..2 only)
                    if 2 <= u <= 7 and (u - 2) // 2 <= 2:
                        sub_unit(u - 2)

                # ---- c_proj helpers ----
                def proj_mm(ps, lhs_sb, col, wslice, start, stop):
                    nc.tensor.matmul(
                        ps[:], lhs_sb[:, col : col + P], wslice,
                        start=start, stop=stop,
                    )

                def projA(t):
                    # session 1: bias + chunks 0,1 (own + partner) -> partial
                    ps = ps_sm.tile([P, 512], F32, name="ps_pA", tag="sm")
                    nc.tensor.matmul(
                        ps[:], ones_row[:, 0:P], bp_row[:], start=True, stop=False
                    )
                    for p in (0, 1):
                        proj_mm(ps, aT_loc, p * S + t * P,
                                wp_sb[:, p * FEAT : (p + 1) * FEAT], False, False)
                        proj_mm(ps, part_sb, p * S + t * P,
                                wp_sb[:, (4 + p) * FEAT : (5 + p) * FEAT],
                                False, p == 1)
                    nc.vector.tensor_copy(
                        out=partial_sb[:, t * FEAT : (t + 1) * FEAT], in_=ps[:]
                    )

                def projB(t, late):
                    # session 2: chunks 2,3; partner ch3 via (b0+b1)*W + mine*(-W)
                    ps = ps_sm.tile([P, 512], F32, name="ps_pB", tag="sm")
                    proj_mm(ps, aT_loc, 2 * S + t * P,
                            wp_sb[:, 2 * FEAT : 3 * FEAT], True, False)
                    proj_mm(ps, part_sb, 2 * S + t * P,
                            wp_sb[:, 6 * FEAT : 7 * FEAT], False, False)
                    proj_mm(ps, aT_loc, 3 * S + t * P,
                            wp_sb[:, 3 * FEAT : 4 * FEAT], False, False)
                    proj_mm(ps, blk0_sb, 3 * S + t * P,
                            wp_sb[:, 7 * FEAT : 8 * FEAT], False, False)
                    proj_mm(ps, blk1_sb, 3 * S + t * P,
                            wp_sb[:, 7 * FEAT : 8 * FEAT], False, False)
                    proj_mm(ps, aT_loc, 3 * S + t * P, wpn3_sb[:, :], False, True)
                    ot = outp.tile([P, FEAT], F32, tag="ot")
                    nc.vector.tensor_tensor(
                        out=ot[:], in0=ps[:],
                        in1=partial_sb[:, t * FEAT : (t + 1) * FEAT], op=ALU.add,
                    )
                    if late:
                        nc.sync.dma_start(out[t * P : (t + 1) * P, :], ot[:])
                    else:
                        nc.gpsimd.dma_start(out[t * P : (t + 1) * P, :], ot[:])

                # ---- schedule ----
                attn_half(0, 0)
                for st in (4, 5, 6, 7):
                    v_tile(st)
                attn_half(0, 1)
                qkT_tile(1)
                qkT_tile(5)
                attn_half(1, 0)
                qkT_tile(2)
                qkT_tile(6)
                attn_half(1, 1)
                qkT_tile(3)
                qkT_tile(7)
                attn_half(2, 0)
                projA(0)
                projA(1)
                attn_half(2, 1)
                projA(2)
                projA(3)
                attn_half(3, 0)
                projA(4)
                projA(5)
                projA(6)
                projA(7)
                attn_half(3, 1)
                for t in range(4):
                    projB(t, late=False)
                for t in range(4, 8):
                    projB(t, late=True)

    nc.finalize()
    return nc


_NC_CACHE = None
_LAST_IN_MAPS = None


def kernel(x, c_attn_w, c_attn_b, c_proj_w, c_proj_b):
    global _NC_CACHE, _LAST_IN_MAPS
    x = np.asarray(x, dtype=np.float32)
    c_attn_w = np.asarray(c_attn_w, dtype=np.float32)
    c_attn_b = np.asarray(c_attn_b, dtype=np.float32)
    c_proj_w = np.asarray(c_proj_w, dtype=np.float32)
    c_proj_b = np.asarray(c_proj_b, dtype=np.float32)
    B = x.shape[0]
    assert x.shape == (B, S, NX)
    bf16 = ml_dtypes.bfloat16

    xTs = [np.ascontiguousarray(x[b].T).astype(bf16) for b in range(B)]
    in_maps = []
    for c in range(8):
        b, hg = c // 2, c % 2
        cols = slice(hg * FEAT, (hg + 1) * FEAT)
        wq = c_attn_w[:, 0 * NX :][:, cols]
        wk = c_attn_w[:, 1 * NX :][:, cols]
        bq = c_attn_b[0 * NX :][cols]
        bk = c_attn_b[1 * NX :][cols]
        bqk = np.concatenate([bq, bk])
        own = slice(hg * FEAT, (hg + 1) * FEAT)
        par = slice((1 - hg) * FEAT, (2 - hg) * FEAT)
        wproj_r = np.concatenate([c_proj_w[own, cols], c_proj_w[par, cols]], axis=0)
        wpn3 = -c_proj_w[par, cols][3 * P : 4 * P, :]
        in_maps.append(
            {
                "xT": xTs[b],
                "wqkv": np.ascontiguousarray(
                    np.concatenate([wq, wk, c_attn_w[:, 2 * NX :][:, cols]], axis=1)
                ).astype(bf16),
                "bqk_t": np.ascontiguousarray(bqk.reshape(8, P).T),
                "bv_r": np.ascontiguousarray(
                    c_attn_b[2 * NX :][cols].reshape(1, FEAT)
                ).astype(bf16),
                "wproj": np.ascontiguousarray(wproj_r).astype(bf16),
                "wpn3": np.ascontiguousarray(wpn3).astype(bf16),
                "bp_r": np.ascontiguousarray(
                    c_proj_b[cols].reshape(1, FEAT)
                ).astype(bf16),
            }
        )

    _LAST_IN_MAPS = in_maps
    if _NC_CACHE is None:
        _NC_CACHE = build()
    res = run_bass_kernel_spmd(_NC_CACHE, in_maps, core_ids=list(range(8)))
    outf = np.empty((B, S, NX), dtype=np.float32)
    for c in range(8):
        b, hg = c // 2, c % 2
        outf[b, :, hg * FEAT : (hg + 1) * FEAT] = res.results[c]["out"]
    return outf


# revision 16
# speedup vs baseline: 1.3755x; 1.3755x over previous
"""Distributed causal multi-head attention block (GPT-2 style) for 8 TRN2 NeuronCores.

Sharding: data-parallel over batch (4 pairs of cores) x tensor-parallel over
heads (2 groups of 8 heads). Core c handles batch c//2, head-group c%2.

v4 reorganization vs v3:
  - ~33 junk matmuls on memset SBUF data at t=0 warm the PE/HAM clock gate
    through the ~9us DMA-bootstrap dead time, so real matmuls run at 2.4 GHz.
  - A tiny warmup AllReduce flushes the collective-stream barrier early.
  - Attention runs per head-PAIR: the two heads of a feature tile (partition
    rows 0:64 / 64:128) issue score matmuls back-to-back at row groups (0,0)
    and (64,0), so both K=64 matmuls stream concurrently through the PE.
  - Causal masking is folded into the scores PSUM via an identity-matmul that
    accumulates -240 above the diagonal BEFORE exp (exp then yields ~0), so
    the DVE is out of the scores->exp->PV critical chain.
  - One ACT exp per (pair, k-block): strided [128, 2, N] over both heads.
  - Head-major schedule: each (pair, half) finishes and immediately stages its
    aT rows for a pair-AllReduce(add); the partner's rows are recovered
    rank-agnostically as (sum - mine) on GpSimd. c_proj contracts own rows
    straight from SBUF and partner rows from the recovered copies, with
    host-reordered Wproj rows [own 512 | partner 512]. Only the last 128KB
    exchange sits near the tail, and its chunk-3 contribution uses the
    (sum*W + mine*(-W)) trick to skip the subtract latency.
  - c_proj per out-tile runs in two PSUM sessions (bias+chunks 0,1 early,
    chunks 2,3 late) joined by an SBUF f32 partial, so early proj matmuls
    interleave with late attention pairs on a 2-bank rotation.
"""

import numpy as np
import ml_dtypes

import concourse.bass as bass
import concourse.mybir as mybir
import concourse.tile as tile
from concourse import bacc
from concourse.bass_utils import run_bass_kernel_spmd
from concourse.masks import make_identity, make_lower_triangular

F32 = mybir.dt.float32
BF16 = mybir.dt.bfloat16
AF = mybir.ActivationFunctionType
ALU = mybir.AluOpType

P = 128
S = 1024          # sequence length
NX = 1024         # model width
D = 64            # head dim
H_LOC = 8         # heads per core
FEAT = H_LOC * D  # 512 local attention features
NKC = NX // P     # 8 contraction chunks
NST = S // P      # 8 sequence tiles
VW = D + 1        # v block width incl. ones column (65)
N_JUNK = 8        # warmup matmuls to open the HAM clock gate early

PAIRS = [[0, 1], [2, 3], [4, 5], [6, 7]]


def build():
    nc = bacc.Bacc(num_devices=8)
    xT = nc.dram_tensor("xT", [NX, S], BF16, kind="ExternalInput")
    wqkv = nc.dram_tensor("wqkv", [NX, 3 * FEAT], BF16, kind="ExternalInput")
    bqk_t = nc.dram_tensor("bqk_t", [P, 8], F32, kind="ExternalInput")
    bv_r = nc.dram_tensor("bv_r", [1, FEAT], BF16, kind="ExternalInput")
    wproj = nc.dram_tensor("wproj", [NX, FEAT], BF16, kind="ExternalInput")
    wpn3 = nc.dram_tensor("wpn3", [P, FEAT], BF16, kind="ExternalInput")
    bp_r = nc.dram_tensor("bp_r", [1, FEAT], BF16, kind="ExternalInput")
    out = nc.dram_tensor("out", [S, FEAT], F32, kind="ExternalOutput")

    with tile.TileContext(nc) as tc:
        with (
            tc.tile_pool(name="pt", bufs=8) as ptp,            # P^T pair blocks
            tc.tile_pool(name="small", bufs=2) as small,       # recip vectors
            tc.tile_pool(name="outp", bufs=3) as outp,         # out f32 tiles
            tc.tile_pool(name="dram", bufs=1, space="DRAM") as dram,
            tc.tile_pool(name="resident", bufs=1) as res,
        ):
            # ---- resident SBUF tensors ----
            xT_all = res.tile([P, NKC * S], BF16, tag="xT_all")
            wqkv_sb = res.tile([P, NKC * 3 * FEAT], BF16, tag="wqkv_sb")
            qkT_all = res.tile([P, 8 * S], BF16, tag="qkT_all")   # qT(0..3)|kT(4..7)
            v_sb = res.tile([P, NST * H_LOC * VW], BF16, tag="v_sb")
            aT_loc = res.tile([P, 4 * S], BF16, tag="aT_loc")     # my 512 feats
            blk0_sb = res.tile([P, 4 * S], BF16, tag="blk0_sb")   # gathered rank0
            blk1_sb = res.tile([P, 4 * S], BF16, tag="blk1_sb")   # gathered rank1
            part_sb = res.tile([P, 4 * S], BF16, tag="part_sb")   # partner rows
            wp_sb = res.tile([P, NKC * FEAT], BF16, tag="wp_sb")  # [own|partner]
            wpn3_sb = res.tile([P, FEAT], BF16, tag="wpn3_sb")    # -W partner ch3
            partial_sb = res.tile([P, 4 * FEAT], F32, tag="partial_sb")
            bias_sb = res.tile([P, 8], F32, tag="bias_sb")
            bv_row = res.tile([1, FEAT], BF16, tag="bv_row")
            bp_row = res.tile([1, FEAT], BF16, tag="bp_row")
            ones_row = res.tile([1, P], BF16, tag="ones_row")
            junk_sb = res.tile([P, 512], BF16, tag="junk_sb")
            iden = res.tile([P, P], BF16, tag="iden")
            mask_add = res.tile([P, P], BF16, tag="mask_add")

            nc.vector.memset(ones_row[:], 1.0)
            nc.vector.memset(junk_sb[:], 0.001)
            nc.vector.memset(v_sb[:], 1.0)
            make_identity(nc, iden[:])
            make_lower_triangular(nc, mask_add[:], val=-240.0, diag=False)

            # ---- input stream. sync queue: x/w chunks (critical path) first,
            # then wproj + wpn3, then collective-sum reloads, then late out
            # tiles. gpsimd queue: biases, warmup-cc staging, per-half
            # broadcasts / staging / subs, early out tiles. ----
            for kc in range(NKC):
                xs = slice(kc * P, (kc + 1) * P)
                if kc < 2:
                    nc.sync.dma_start(
                        wqkv_sb[:, kc * 3 * FEAT : kc * 3 * FEAT + 640],
                        wqkv[xs, 0:640],
                    )
                    nc.sync.dma_start(
                        xT_all[:, kc * S : kc * S + 512], xT[xs, 0:512]
                    )
                    nc.sync.dma_start(
                        xT_all[:, kc * S + 512 : (kc + 1) * S], xT[xs, 512:1024]
                    )
                    nc.sync.dma_start(
                        wqkv_sb[:, kc * 3 * FEAT + 640 : (kc + 1) * 3 * FEAT],
                        wqkv[xs, 640:1536],
                    )
                else:
                    nc.sync.dma_start(xT_all[:, kc * S : (kc + 1) * S], xT[xs, :])
                    nc.sync.dma_start(
                        wqkv_sb[:, kc * 3 * FEAT : (kc + 1) * 3 * FEAT],
                        wqkv[xs, :],
                    )
            # warmup collective first on the gpsimd queue: DRAM->DRAM staging
            # with no compute deps, so the cc-stream barrier runs during boot
            cc_w_in = dram.tile([1, 8], F32, name="cc_w_in")
            cc_w_out = dram.tile([2, 8], F32, name="cc_w_out")
            nc.gpsimd.dma_start(cc_w_in[:, :], bqk_t[0:1, 0:8])
            nc.gpsimd.collective_compute(
                "AllGather", ALU.bypass, replica_groups=PAIRS,
                ins=[cc_w_in[:].opt()], outs=[cc_w_out[:].opt()],
            )
            nc.gpsimd.dma_start(bias_sb[:], bqk_t[:, :])
            nc.gpsimd.dma_start(bv_row[:], bv_r[:, :])
            nc.gpsimd.dma_start(bp_row[:], bp_r[:, :])
            for fc in range(NKC):
                nc.sync.dma_start(
                    wp_sb[:, fc * FEAT : (fc + 1) * FEAT],
                    wproj[fc * P : (fc + 1) * P, :],
                )
            nc.sync.dma_start(wpn3_sb[:], wpn3[:, :])

            # ---- collective staging: one big qh0 gather, 3 qh1 parts ----
            cc_inA = dram.tile([4 * P, 512], BF16, name="cc_inA")
            cc_outA = dram.tile([8 * P, 512], BF16, name="cc_outA")
            B_PARTS = [[0, 1], [2], [3]]
            cc_inB = [
                dram.tile([len(ps_) * P, 512], BF16, name=f"cc_inB{i}")
                for i, ps_ in enumerate(B_PARTS)
            ]
            cc_outB = [
                dram.tile([2 * len(ps_) * P, 512], BF16, name=f"cc_outB{i}")
                for i, ps_ in enumerate(B_PARTS)
            ]

            # ---- qkv group helpers ----
            def qk_mm(ps, ft, half, kc):
                nc.tensor.matmul(
                    ps[:],
                    wqkv_sb[:, kc * 3 * FEAT + ft * P : kc * 3 * FEAT + (ft + 1) * P],
                    xT_all[:, kc * S + half * 512 : kc * S + (half + 1) * 512],
                    start=(kc == 0),
                    stop=(kc == NKC - 1),
                )

            def qk_consume(ps, ft, half):
                nc.vector.tensor_scalar_add(
                    out=qkT_all[:, ft * S + half * 512 : ft * S + (half + 1) * 512],
                    in0=ps[:],
                    scalar1=bias_sb[:, ft : ft + 1],
                )

            def v_bias(ps):
                nc.tensor.matmul(ps[:], ones_row[:, 0:P], bv_row[:], start=True, stop=False)

            def v_mm(ps, st, kc):
                nc.tensor.matmul(
                    ps[:],
                    xT_all[:, kc * S + st * P : kc * S + (st + 1) * P],
                    wqkv_sb[:, kc * 3 * FEAT + 1024 : kc * 3 * FEAT + 1536],
                    start=False,
                    stop=(kc == NKC - 1),
                )

            def v_consume(ps, st):
                base = st * H_LOC * VW
                dst = v_sb[:, base : base + H_LOC * VW].rearrange(
                    "p (h w) -> p h w", h=H_LOC
                )[:, :, 0:D]
                src = ps[:].rearrange("p (h d) -> p h d", h=H_LOC)
                nc.vector.tensor_copy(out=dst, in_=src)

            # ---- wave 1: junk warmup + 8 groups fed in DMA-arrival order ----
            W1_QK = [(0, 0), (0, 1), (4, 0), (4, 1)]
            W1_V = [0, 1, 2, 3]
            with tc.tile_pool(name="ps_w1", bufs=8, space="PSUM") as psw:
                junk_ps = psw.tile([P, 512], F32, name="junk_ps", tag="w1")
                for _ in range(N_JUNK):
                    nc.tensor.matmul(
                        junk_ps[:], junk_sb[:, 0:P], junk_sb[:, 0:512],
                        start=True, stop=True,
                    )
                w1ps = {}
                for ft, half in W1_QK:
                    w1ps[("qk", ft, half)] = psw.tile(
                        [P, 512], F32, name=f"w1qk{ft}{half}", tag="w1"
                    )
                for st in W1_V:
                    ps = psw.tile([P, 512], F32, name=f"w1v{st}", tag="w1")
                    w1ps[("v", st)] = ps
                    v_bias(ps)
                for kc in range(NKC):
                    for ft, half in [(0, 0), (4, 0), (0, 1), (4, 1)]:
                        qk_mm(w1ps[("qk", ft, half)], ft, half, kc)
                    for st in W1_V:
                        v_mm(w1ps[("v", st)], st, kc)
                for ft, half in W1_QK:
                    qk_consume(w1ps[("qk", ft, half)], ft, half)
                for st in W1_V:
                    v_consume(w1ps[("v", st)], st)

            # ---- attention + remaining qkv + proj, interleaved ----
            with (
                tc.tile_pool(name="ps_sc", bufs=2, space="PSUM") as ps_sc,
                tc.tile_pool(name="ps_pa", bufs=1, space="PSUM") as ps_pa,
                tc.tile_pool(name="ps_sm", bufs=2, space="PSUM") as ps_sm,
            ):
                def qkT_tile(ft):
                    for half in range(2):
                        ps = ps_sm.tile([P, 512], F32, name="ps_qk", tag="sm")
                        for kc in range(NKC):
                            qk_mm(ps, ft, half, kc)
                        qk_consume(ps, ft, half)

                def v_tile(st):
                    ps = ps_sm.tile([P, 512], F32, name="ps_v", tag="sm")
                    v_bias(ps)
                    for kc in range(NKC):
                        v_mm(ps, st, kc)
                    v_consume(ps, st)

                def recover(p, qh):
                    # partner chunk p half qh = (block0 + block1) - my rows, DVE
                    col = p * S + qh * 512
                    nc.vector.tensor_tensor(
                        out=part_sb[:, col : col + 512],
                        in0=blk0_sb[:, col : col + 512],
                        in1=blk1_sb[:, col : col + 512],
                        op=ALU.add,
                    )
                    nc.vector.tensor_tensor(
                        out=part_sb[:, col : col + 512],
                        in0=part_sb[:, col : col + 512],
                        in1=aT_loc[:, col : col + 512],
                        op=ALU.subtract,
                    )

                def stage_A():
                    # all four chunks' qh0 halves -> 512KB pair AllGather
                    for p in range(4):
                        nc.gpsimd.dma_start(
                            cc_inA[p * P : (p + 1) * P, :],
                            aT_loc[:, p * S : p * S + 512],
                        )
                    nc.gpsimd.collective_compute(
                        "AllGather", ALU.bypass, replica_groups=PAIRS,
                        ins=[cc_inA[:].opt()], outs=[cc_outA[:].opt()],
                    )

                def reload_A():
                    for p in range(4):
                        nc.sync.dma_start(
                            blk0_sb[:, p * S : p * S + 512],
                            cc_outA[p * P : (p + 1) * P, :],
                        )
                        nc.sync.dma_start(
                            blk1_sb[:, p * S : p * S + 512],
                            cc_outA[(4 + p) * P : (5 + p) * P, :],
                        )

                def stage_B(i):
                    ps_ = B_PARTS[i]
                    for k, p in enumerate(ps_):
                        nc.gpsimd.dma_start(
                            cc_inB[i][k * P : (k + 1) * P, :],
                            aT_loc[:, p * S + 512 : (p + 1) * S],
                        )
                    nc.gpsimd.collective_compute(
                        "AllGather", ALU.bypass, replica_groups=PAIRS,
                        ins=[cc_inB[i][:].opt()], outs=[cc_outB[i][:].opt()],
                    )

                def reload_B(i):
                    ps_ = B_PARTS[i]
                    n = len(ps_)
                    for k, p in enumerate(ps_):
                        nc.sync.dma_start(
                            blk0_sb[:, p * S + 512 : (p + 1) * S],
                            cc_outB[i][k * P : (k + 1) * P, :],
                        )
                        nc.sync.dma_start(
                            blk1_sb[:, p * S + 512 : (p + 1) * S],
                            cc_outB[i][(n + k) * P : (n + k + 1) * P, :],
                        )

                def attn_half(pair, qh):
                    nj = 4 * qh + 4
                    kcol = (4 + pair) * S
                    qbase = pair * S + qh * 512
                    u = pair * 2 + qh
                    pt_blocks = []
                    for j in range(nj):
                        dloc = j - 4 * qh
                        coff = max(dloc, 0) * P
                        diag = dloc >= 0
                        ps = ps_sc.tile([P, 1024], F32, name="ps_s", tag="sc")
                        ptb = ptp.tile([P, 1024], BF16, name="ptb", tag="pt")
                        for hh in range(2):
                            nc.tensor.matmul(
                                ps[:, hh * 512 + coff : hh * 512 + 512],
                                qkT_all[hh * D : (hh + 1) * D,
                                        kcol + j * P : kcol + (j + 1) * P],
                                qkT_all[hh * D : (hh + 1) * D,
                                        qbase + coff : qbase + 512],
                                start=True,
                                stop=not diag,
                            )
                        if diag:
                            for hh in range(2):
                                nc.tensor.matmul(
                                    ps[:, hh * 512 + coff : hh * 512 + coff + P],
                                    iden[:, 0:P],
                                    mask_add[:, 0:P],
                                    start=False,
                                    stop=True,
                                    skip_group_check=True,
                                )
                        pr = ps[:].rearrange("p (b n) -> p b n", b=2)[:, :, coff:512]
                        tr = ptb[:].rearrange("p (b n) -> p b n", b=2)[:, :, coff:512]
                        nc.scalar.activation(out=tr, in_=pr, func=AF.Exp, scale=0.125)
                        pt_blocks.append((ptb, coff))
                    psa = ps_pa.tile([P, 1024], F32, name="psa", tag="pa")
                    for j, (ptb, coff) in enumerate(pt_blocks):
                        for hh in range(2):
                            h = 2 * pair + hh
                            nc.tensor.matmul(
                                psa[:VW, hh * 512 + coff : hh * 512 + 512],
                                v_sb[:, j * H_LOC * VW + h * VW
                                     : j * H_LOC * VW + (h + 1) * VW],
                                ptb[:, hh * 512 + coff : hh * 512 + 512],
                                start=(j == 0),
                                stop=(j == nj - 1),
                            )
                    # normalize: recip of denominator row, broadcast, scale
                    db = small.tile([1, 1024], F32, tag="db")
                    nc.vector.tensor_copy(out=db[:], in_=psa[D : D + 1, 0:1024])
                    rc = small.tile([1, 1024], F32, tag="rc")
                    nc.vector.reciprocal_approx_fast(rc[:], db[:])
                    bcs = small.tile([D, 1024], F32, tag="bcs")
                    nc.gpsimd.partition_broadcast(bcs[:], rc[:])
                    acol = pair * S + qh * 512
                    for hh in range(2):
                        nc.vector.tensor_tensor(
                            out=aT_loc[hh * D : (hh + 1) * D, acol : acol + 512],
                            in0=bcs[:, hh * 512 : (hh + 1) * 512],
                            in1=psa[0:D, hh * 512 : (hh + 1) * 512],
                            op=ALU.mult,
                        )


                # ---- c_proj helpers ----
                def proj_mm(ps, lhs_sb, col, wslice, start, stop):
                    nc.tensor.matmul(
                        ps[:], lhs_sb[:, col : col + P], wslice,
                        start=start, stop=stop,
                    )

                def proj_full(t):
                    # out rows t*128 (qh0): one session over all 8 chunks
                    ps = ps_sm.tile([P, 512], F32, name="ps_pf", tag="sm")
                    nc.tensor.matmul(
                        ps[:], ones_row[:, 0:P], bp_row[:], start=True, stop=False
                    )
                    for p in range(4):
                        proj_mm(ps, aT_loc, p * S + t * P,
                                wp_sb[:, p * FEAT : (p + 1) * FEAT], False, False)
                        proj_mm(ps, part_sb, p * S + t * P,
                                wp_sb[:, (4 + p) * FEAT : (5 + p) * FEAT],
                                False, p == 3)
                    ot = outp.tile([P, FEAT], F32, tag="ot")
                    nc.vector.tensor_copy(out=ot[:], in_=ps[:])
                    nc.gpsimd.dma_start(out[t * P : (t + 1) * P, :], ot[:])

                def projA2(t):
                    # out rows t*128 (qh1), session 1: bias + chunks 0..2
                    ps = ps_sm.tile([P, 512], F32, name="ps_pA", tag="sm")
                    nc.tensor.matmul(
                        ps[:], ones_row[:, 0:P], bp_row[:], start=True, stop=False
                    )
                    for p in range(3):
                        proj_mm(ps, aT_loc, p * S + t * P,
                                wp_sb[:, p * FEAT : (p + 1) * FEAT], False, False)
                        proj_mm(ps, part_sb, p * S + t * P,
                                wp_sb[:, (4 + p) * FEAT : (5 + p) * FEAT],
                                False, p == 2)
                    nc.vector.tensor_copy(
                        out=partial_sb[:, (t - 4) * FEAT : (t - 3) * FEAT], in_=ps[:]
                    )

                def projB2(t):
                    # session 2: chunk 3; partner via (b0+b1)*W + mine*(-W)
                    ps = ps_sm.tile([P, 512], F32, name="ps_pB", tag="sm")
                    proj_mm(ps, aT_loc, 3 * S + t * P,
                            wp_sb[:, 3 * FEAT : 4 * FEAT], True, False)
                    proj_mm(ps, blk0_sb, 3 * S + t * P,
                            wp_sb[:, 7 * FEAT : 8 * FEAT], False, False)
                    proj_mm(ps, blk1_sb, 3 * S + t * P,
                            wp_sb[:, 7 * FEAT : 8 * FEAT], False, False)
                    proj_mm(ps, aT_loc, 3 * S + t * P, wpn3_sb[:, :], False, True)
                    ot = outp.tile([P, FEAT], F32, tag="ot")
                    nc.vector.tensor_tensor(
                        out=ot[:], in0=ps[:],
                        in1=partial_sb[:, (t - 4) * FEAT : (t - 3) * FEAT],
                        op=ALU.add,
                    )
                    nc.sync.dma_start(out[t * P : (t + 1) * P, :], ot[:])

                # ---- schedule: qh0 sweep, big gather, qh1 sweep + proj ----
                attn_half(0, 0)
                qkT_tile(1)
                qkT_tile(5)
                attn_half(1, 0)
                qkT_tile(2)
                qkT_tile(6)
                attn_half(2, 0)
                qkT_tile(3)
                qkT_tile(7)
                attn_half(3, 0)
                stage_A()
                for st in (4, 5, 6, 7):
                    v_tile(st)
                reload_A()
                attn_half(0, 1)
                for p in range(4):
                    recover(p, 0)
                attn_half(1, 1)
                stage_B(0)
                proj_full(0)
                proj_full(1)
                attn_half(2, 1)
                stage_B(1)
                reload_B(0)
                proj_full(2)
                proj_full(3)
                recover(0, 1)
                recover(1, 1)
                attn_half(3, 1)
                stage_B(2)
                reload_B(1)
                reload_B(2)
                recover(2, 1)
                for t in range(4, 8):
                    projA2(t)
                for t in range(4, 8):
                    projB2(t)

    nc.finalize()
    return nc


_NC_CACHE = None
_LAST_IN_MAPS = None


def kernel(x, c_attn_w, c_attn_b, c_proj_w, c_proj_b):
    global _NC_CACHE, _LAST_IN_MAPS
    x = np.asarray(x, dtype=np.float32)
    c_attn_w = np.asarray(c_attn_w, dtype=np.float32)
    c_attn_b = np.asarray(c_attn_b, dtype=np.float32)
    c_proj_w = np.asarray(c_proj_w, dtype=np.float32)
    c_proj_b = np.asarray(c_proj_b, dtype=np.float32)
    B = x.shape[0]
    assert x.shape == (B, S, NX)
    bf16 = ml_dtypes.bfloat16

    xTs = [np.ascontiguousarray(x[b].T).astype(bf16) for b in range(B)]
    in_maps = []
    for c in range(8):
        b, hg = c // 2, c % 2
        cols = slice(hg * FEAT, (hg + 1) * FEAT)
        wq = c_attn_w[:, 0 * NX :][:, cols]
        wk = c_attn_w[:, 1 * NX :][:, cols]
        bq = c_attn_b[0 * NX :][cols]
        bk = c_attn_b[1 * NX :][cols]
        bqk = np.concatenate([bq, bk])
        own = slice(hg * FEAT, (hg + 1) * FEAT)
        par = slice((1 - hg) * FEAT, (2 - hg) * FEAT)
        wproj_r = np.concatenate([c_proj_w[own, cols], c_proj_w[par, cols]], axis=0)
        wpn3 = -c_proj_w[par, cols][3 * P : 4 * P, :]
        in_maps.append(
            {
                "xT": xTs[b],
                "wqkv": np.ascontiguousarray(
                    np.concatenate([wq, wk, c_attn_w[:, 2 * NX :][:, cols]], axis=1)
                ).astype(bf16),
                "bqk_t": np.ascontiguousarray(bqk.reshape(8, P).T),
                "bv_r": np.ascontiguousarray(
                    c_attn_b[2 * NX :][cols].reshape(1, FEAT)
                ).astype(bf16),
                "wproj": np.ascontiguousarray(wproj_r).astype(bf16),
                "wpn3": np.ascontiguousarray(wpn3).astype(bf16),
                "bp_r": np.ascontiguousarray(
                    c_proj_b[cols].reshape(1, FEAT)
                ).astype(bf16),
            }
        )

    _LAST_IN_MAPS = in_maps
    if _NC_CACHE is None:
        _NC_CACHE = build()
    res = run_bass_kernel_spmd(_NC_CACHE, in_maps, core_ids=list(range(8)))
    outf = np.empty((B, S, NX), dtype=np.float32)
    for c in range(8):
        b, hg = c // 2, c % 2
        outf[b, :, hg * FEAT : (hg + 1) * FEAT] = res.results[c]["out"]
    return outf
